# revision 1
# baseline (speedup 1.0000x reference)
"""Trainium2 Bass kernel for DifferentiableSupergraphDynamics.

Computation:
    edge_w = where(learn_mask, tanh(theta), sign*conf) * delay      [E]
    msgs   = x[:, src] * edge_w                                     [B, E]
    agg    = scatter_add(msgs -> dst)                               [B, N]
    rate   = base_rate * exp(rate_log_scale)                        [N]
    drive  = tanh(agg + bias)
    x_next = clip(x + DT * rate * (drive*cap - x), 0, cap)

Sharding: destination nodes are dealt round-robin (by total in-degree
rank) across the 8 cores; every edge lives on its destination's core, so
no cross-core collective is needed.

Per-core edge phase: edges are split into (up to) 4 "structures" by
source-node range (32768 rows each, so dma_gather's int16 indices can
address the x table). Each structure is a padded CSR over the core's
nodes sorted by that structure's in-degree: node groups of 128
partitions padded to the group max degree D. The x-row gather for all of
a structure's slots is done with the vectorized SWDGE dma_gather ucode
(one 64B descriptor per slot, round-robin over the 4 SWDGE queues), the
weighted per-node reduction is a strided Vector-engine tensor_reduce,
and the 4 per-structure partial aggregates are merged into structure-0's
node order with unique-index dma_scatter_add through HBM.
"""

import numpy as np

import concourse.bass as bass
import concourse.bacc as bacc
import concourse.mybir as mybir
import concourse.tile as tile
from concourse.bass_utils import run_bass_kernel_spmd

P = 128
NCORES = 8
DT = 0.1
SRC_CHUNK = 32768          # dma_gather int16 index reach
GATHER_CALL = 8192         # SWDGE ring capacity per call
XBF16 = False              # f32 x rows (64B descriptors); bf16 trips clip-boundary rel err
ROWE = 64                  # x-table row stride: 256B (dma_gather req)
XROW = 128 if XBF16 else 64

F32 = mybir.dt.float32
I16 = mybir.dt.int16
I8 = mybir.dt.int8


def _wrap_idx(flat, call):
    """Lay a flat int16 index list out in the SWDGE wrapped layout:
    per call of `call` indices, index j at [j%16, j//16]; 32-partition
    replicated (descriptor-gen runs on two Q7 cores)."""
    n = len(flat)
    ncall = (n + call - 1) // call
    pad = ncall * call - n
    if pad:
        flat = np.concatenate([flat, np.zeros(pad, flat.dtype)])
    cols = np.concatenate(
        [flat[k * call:(k + 1) * call].reshape(call // 16, 16).T
         for k in range(ncall)], axis=1)          # [16, ncall*call/16]
    return np.concatenate([cols] * 8, axis=0)     # [128, ...]


# ---------------------------------------------------------------------------
# Host-side data preparation
# ---------------------------------------------------------------------------

def _prep(x, theta, bias, ratelog, baserate, cap, sign, conf, delay, src, dst,
          mask, n_cores):
    B, N = x.shape
    E = src.shape[0]

    src = np.asarray(src).astype(np.int64)
    dst = np.asarray(dst).astype(np.int64)
    theta = np.asarray(theta, dtype=np.float32)
    sign = np.asarray(sign, dtype=np.float32)
    conf = np.asarray(conf, dtype=np.float32)
    delay = np.asarray(delay, dtype=np.float32)
    mask8 = np.asarray(mask).astype(np.int8)

    deg = np.bincount(dst, minlength=N)
    order = np.argsort(-deg, kind="stable")
    npc = (N + n_cores - 1) // n_cores
    G = (npc + P - 1) // P
    nper = G * P                                   # nodes per core (padded)

    rank_of = np.empty(N, dtype=np.int64)
    rank_of[order] = np.arange(N)
    core_of = rank_of % n_cores                    # node -> core
    pos_of = rank_of // n_cores                    # node -> position in core

    n_pad = ((N + ROWE - 1) // ROWE) * ROWE
    nq = (n_pad + SRC_CHUNK - 1) // SRC_CHUNK     # structures
    q_of = src // SRC_CHUNK                        # edge -> structure

    # per (core, structure) in-degree
    edge_core = core_of[dst]
    edge_pos = pos_of[dst]
    degq = np.zeros((n_cores, nper, nq), dtype=np.int64)
    np.add.at(degq, (edge_core, edge_pos, q_of), 1)

    # shared-over-cores placement per structure: within each core sort
    # positions by degq desc; group windows of 128; D = max over cores.
    D = np.zeros((nq, G), dtype=np.int64)
    ordq = np.zeros((n_cores, nq, nper), dtype=np.int64)   # row j -> position
    invq = np.zeros((n_cores, nq, nper), dtype=np.int64)   # position -> row j
    for q in range(nq):
        for c in range(n_cores):
            o = np.argsort(-degq[c, :, q], kind="stable")
            ordq[c, q] = o
            invq[c, q, o] = np.arange(nper)
            dm = degq[c, o, q].reshape(G, P).max(axis=1)
            D[q] = np.maximum(D[q], dm)
    D[0] = np.maximum(D[0], 1)       # canonical layout covers all nodes
    S = np.zeros((nq, G + 1), dtype=np.int64)
    S[:, 1:] = np.cumsum(D, axis=1)
    F = S[:, -1]                                   # cols per structure
    Gact = np.array([int((D[q] > 0).sum()) for q in range(nq)])

    # --- edge slot assignment ---
    # edge -> (core, structure, row=invq, occurrence within (node,structure))
    eord = np.lexsort((src, dst))                  # group by dst, then src q
    ec = edge_core[eord]
    ep = edge_pos[eord]
    eq = q_of[eord]
    # occurrence counter within (dst, q): edges sorted by (dst, q)
    key_change = np.ones(E, dtype=bool)
    key_change[1:] = (dst[eord][1:] != dst[eord][:-1]) | (eq[1:] != eq[:-1])
    run_id = np.cumsum(key_change) - 1
    run_starts = np.flatnonzero(key_change)
    occ = np.arange(E) - run_starts[run_id]

    row = invq[ec, eq, ep]                         # row index in structure
    g = row // P
    pp = row % P
    col = S[eq, g] + occ
    # slot linear index within (core, structure): i = pp + 128*col
    slot_i = pp + P * col

    # per (core, structure) arrays
    ins = []
    FT = int(F.sum())                              # total cols, all structures
    Scol = np.zeros(nq + 1, dtype=np.int64)
    Scol[1:] = np.cumsum(F)

    # params laid out [P, FT] per core (slot (q,p,col) -> [p, Scol[q]+col])
    par_shape = (n_cores, P, FT)
    thetaA = np.zeros(par_shape, np.float32)
    signA = np.zeros(par_shape, np.float32)
    confA = np.zeros(par_shape, np.float32)
    delayA = np.zeros(par_shape, np.float32)
    maskA = np.zeros(par_shape, np.int8)
    pidx = (ec, pp, Scol[eq] + col)
    thetaA[pidx] = theta[eord]
    signA[pidx] = sign[eord]
    confA[pidx] = conf[eord]
    delayA[pidx] = delay[eord]
    maskA[pidx] = mask8[eord]

    # gather index lists (wrapped) per core: concat over structures of
    # the per-structure slot-order index list (idx relative to q base)
    srcrel = (src[eord] - eq * SRC_CHUNK).astype(np.int16)
    gidx = []
    ncalls = np.zeros(nq, dtype=np.int64)
    for c in range(n_cores):
        parts = []
        for q in range(nq):
            tot = int(F[q]) * P
            a = np.zeros(tot, np.int16)
            selq = (ec == c) & (eq == q)
            a[slot_i[selq]] = srcrel[selq]
            parts.append(_wrap_idx(a, GATHER_CALL))
            ncalls[q] = (tot + GATHER_CALL - 1) // GATHER_CALL
        gidx.append(np.concatenate(parts, axis=1))
    gidx_cols = gidx[0].shape[1]

    # canonical placement = structure 0's; scatter index for structures
    # 1..nq-1: input position i = p + 128*g -> canonical slot p0*G + g0
    sidx = []
    for c in range(n_cores):
        parts = []
        for q in range(1, nq):
            node_pos = ordq[c, q]                  # row j -> position
            crow = invq[c, 0, node_pos]            # canonical row of node
            canon = (crow % P) * G + (crow // P)   # partition-major slot id
            a = np.zeros(nper, np.int16)
            jj = np.arange(nper)
            a[(jj % P) + P * (jj // P)] = canon.astype(np.int16)
            nact = int(Gact[q]) * P
            half = min((G // 2) * P, nact)
            parts.append(_wrap_idx(a[:half], half))
            if nact > half:
                parts.append(_wrap_idx(a[half:nact], nact - half))
        sidx.append(np.concatenate(parts, axis=1) if parts else
                    np.zeros((128, 16), np.int16))
    sidx_cols = sidx[0].shape[1]

    # node params in canonical placement [P, G]
    def node_arr(vals, fill):
        a = np.full((n_cores, P, G), fill, dtype=np.float32)
        for c in range(n_cores):
            node_pos = ordq[c, 0]                  # canonical row j -> pos
            rank = node_pos * n_cores + c          # position -> rank
            ok = rank < N
            nd = order[np.minimum(rank, N - 1)]
            v = np.where(ok, vals[nd], fill).astype(np.float32)
            a[c].reshape(-1)[(np.arange(nper) % P) * G +
                             (np.arange(nper) // P)] = np.where(
                                 ok, v, fill)
        return a

    biasA = node_arr(np.asarray(bias), 0.0)
    ratelogA = node_arr(np.asarray(ratelog), 0.0)
    baserateA = node_arr(np.asarray(baserate), 0.0)
    capA = node_arr(np.asarray(cap), 1.0)

    import ml_dtypes
    xdt = ml_dtypes.bfloat16 if XBF16 else np.float32
    xT4 = np.zeros((n_pad, XROW), xdt)
    xT4[:N, :B] = np.asarray(x, dtype=np.float32).T.astype(xdt)

    xTf = np.asarray(x, dtype=np.float32).T
    xownA = np.zeros((n_cores, P, G, B), np.float32)
    node_ids = np.zeros((n_cores, P, G), np.int64)
    for c in range(n_cores):
        node_pos = ordq[c, 0]
        rank = node_pos * n_cores + c
        ok = rank < N
        nd = np.where(ok, order[np.minimum(rank, N - 1)], -1)
        jj = np.arange(nper)
        pcol = (jj % P, jj // P)
        node_ids[c][pcol] = nd
        xownA[c][pcol[0], pcol[1], :] = np.where(
            ok[:, None], xTf[np.maximum(nd, 0), :], 0.0)

    for c in range(n_cores):
        ins.append({
            "xT4": xT4,
            "gidx": gidx[c],
            "sidx": sidx[c],
            "theta": thetaA[c],
            "sgn": signA[c],
            "conf": confA[c],
            "delay": delayA[c],
            "maskf": maskA[c],
            "bias": biasA[c],
            "ratelog": ratelogA[c],
            "baserate": baserateA[c],
            "cap": capA[c],
            "xown": xownA[c].reshape(P, G * B),
        })
    plan = dict(B=B, N=N, G=G, nq=nq, D=D, S=S, F=F, Scol=Scol, Gact=Gact,
                n_pad=n_pad, gidx_cols=gidx_cols, sidx_cols=sidx_cols,
                node_ids=node_ids)
    return ins, plan


def _assemble(results, plan):
    B, N, G = plan["B"], plan["N"], plan["G"]
    out = np.empty((B, N), dtype=np.float32)
    for ci, res in enumerate(results):
        o = res["out"].reshape(P, G, B)
        nid = plan["node_ids"][ci]
        ok = nid >= 0
        out[:, nid[ok]] = o[ok].T
    return out


# ---------------------------------------------------------------------------
# Device kernel
# ---------------------------------------------------------------------------

def _raw_dma_gather(g, out_ap, in_ap, idxs_ap, num_idxs, elem_size, elem_step,
                    queue_num):
    stride_bytes = elem_step * mybir.dt.size(in_ap.dtype)
    return g.add_instruction(
        mybir.InstDMAGatherAnt(
            name=g.bass.get_next_instruction_name(),
            ins=[*g.lower_ap_dma(in_ap, for_custom_bir_dma=True),
                 g.lower_ap(idxs_ap), g.lower_val_access(g.to_reg(num_idxs))],
            outs=[g.lower_ap(out_ap)],
            transpose=False, num_idxs=num_idxs, elem_size=elem_size,
            stride_bytes_256=stride_bytes // 256, gen_mode=0,
            single_packet=False, queue_num=queue_num,
            sbuf_tokens_per_rank=0, sbuf_free_dim_per_rank=0,
            sbuf_free_dim_pad_per_rank=0, sbuf_byte_offset=0))


def _equal_d_runs(D, g0, g1):
    runs = []
    a = g0
    while a < g1:
        b = a + 1
        while b < g1 and D[b] == D[a]:
            b += 1
        runs.append((a, b, int(D[a])))
        a = b
    return runs


def build(B, N, G, nq, D, S, F, Scol, n_pad, gidx_cols, sidx_cols,
          Gact=None, node_ids=None, enable_asserts=False, loop_r=None):
    if Gact is None:
        Gact = np.array([G] * nq)
    FT = int(Scol[-1])
    nc = bacc.Bacc("TRN2", target_bir_lowering=False, debug=False,
                   enable_asserts=enable_asserts, num_swdge_queues=4)

    XDT = mybir.dt.bfloat16 if XBF16 else F32
    xT4 = nc.dram_tensor("xT4", [n_pad, XROW], XDT, kind="ExternalInput")
    giD = nc.dram_tensor("gidx", [128, gidx_cols], I16, kind="ExternalInput")
    siD = nc.dram_tensor("sidx", [128, sidx_cols], I16, kind="ExternalInput")
    thD = nc.dram_tensor("theta", [P, FT], F32, kind="ExternalInput")
    sgD = nc.dram_tensor("sgn", [P, FT], F32, kind="ExternalInput")
    cfD = nc.dram_tensor("conf", [P, FT], F32, kind="ExternalInput")
    dlD = nc.dram_tensor("delay", [P, FT], F32, kind="ExternalInput")
    mkD = nc.dram_tensor("maskf", [P, FT], I8, kind="ExternalInput")
    biD = nc.dram_tensor("bias", [P, G], F32, kind="ExternalInput")
    rlD = nc.dram_tensor("ratelog", [P, G], F32, kind="ExternalInput")
    brD = nc.dram_tensor("baserate", [P, G], F32, kind="ExternalInput")
    cpD = nc.dram_tensor("cap", [P, G], F32, kind="ExternalInput")
    xoD = nc.dram_tensor("xown", [P, G * B], F32, kind="ExternalInput")
    outD = nc.dram_tensor("out", [P, G * B], F32, kind="ExternalOutput")
    # partial-agg merge buffers (zero-initialized by the runtime)
    pagg = [nc.dram_tensor(f"pagg{q}", [G * P, ROWE], F32,
                           kind="ExternalOutput") for q in range(1, nq)]

    Tanh = mybir.ActivationFunctionType.Tanh
    Exp = mybir.ActivationFunctionType.Exp

    qrr = [0]

    def next_q():
        qrr[0] = (qrr[0] + 1) % 4
        return qrr[0]

    import contextlib
    with tile.TileContext(nc) as tc:
        with (
            tc.tile_pool(name="persist", bufs=1) as ppool,
            tc.tile_pool(name="work", bufs=2) as wp,
            tc.tile_pool(name="msgs", bufs=2) as mp,
        ):
          with (tc.For_i(0, loop_r, 1) if loop_r else
                contextlib.nullcontext()):
            agg0 = ppool.tile([P, G * B], F32, tag="agg0")
            aggq_tiles = []

            qorder = list(range(1, nq)) + [0]
            gidx_bases = np.zeros(nq + 1, dtype=np.int64)
            sidx_bases = np.zeros(nq, dtype=np.int64)
            sb = 0
            for q in range(nq):
                tot = int(F[q]) * P
                ncall = (tot + GATHER_CALL - 1) // GATHER_CALL if tot else 0
                gidx_bases[q + 1] = gidx_bases[q] + ncall * (GATHER_CALL // 16)
                if q >= 1:
                    sidx_bases[q] = sb
                    nact = int(Gact[q]) * P
                    sb += (nact + 15) // 16
            for q in qorder:
                Fq = int(F[q])
                TOTq = Fq * P
                if TOTq == 0:
                    continue
                ncall = (TOTq + GATHER_CALL - 1) // GATHER_CALL
                icols = ncall * (GATHER_CALL // 16)
                gidx_t = wp.tile([128, icols], I16, tag="gidx")
                gb = int(gidx_bases[q])
                nc.sync.dma_start(
                    out=gidx_t[:],
                    in_=giD[:, gb:gb + icols])

                th = wp.tile([P, Fq], F32, tag="th")
                sg = wp.tile([P, Fq], F32, tag="sg")
                cf = wp.tile([P, Fq], F32, tag="cf")
                dl = wp.tile([P, Fq], F32, tag="dl")
                mk = wp.tile([P, Fq], I8, tag="mk")
                c0, c1 = int(Scol[q]), int(Scol[q + 1])
                nc.sync.dma_start(out=th[:], in_=thD[:, c0:c1])
                nc.sync.dma_start(out=sg[:], in_=sgD[:, c0:c1])
                nc.sync.dma_start(out=cf[:], in_=cfD[:, c0:c1])
                nc.sync.dma_start(out=dl[:], in_=dlD[:, c0:c1])
                nc.sync.dma_start(out=mk[:], in_=mkD[:, c0:c1])

                t = wp.tile([P, Fq], F32, tag="t")
                w = wp.tile([P, Fq], F32, tag="w")
                nc.scalar.activation(t[:], th[:], Tanh)
                nc.vector.tensor_mul(w[:], sg[:], cf[:])
                nc.vector.copy_predicated(w[:], mk[:], t[:])
                nc.vector.tensor_mul(w[:], w[:], dl[:])

                msgs = mp.tile([P, Fq * B], F32, tag="msgs")
                m3 = msgs[:].rearrange("p (s b) -> p s b", b=B)
                base = q * SRC_CHUNK
                in_ap = xT4[base:min(base + SRC_CHUNK, n_pad), :B]
                if XBF16:
                    msgsr = mp.tile([P, Fq * B], XDT, tag="msgsr")
                    gdst = msgsr[:].rearrange("p (s b) -> p s b", b=B)
                else:
                    gdst = m3
                for k in range(ncall):
                    i0 = k * GATHER_CALL
                    ni = min(GATHER_CALL, TOTq - i0)
                    _raw_dma_gather(
                        nc.gpsimd,
                        gdst[:, i0 // P:(i0 + ni) // P, :],
                        in_ap,
                        gidx_t[:, k * (GATHER_CALL // 16):
                               k * (GATHER_CALL // 16) + (ni + 15) // 16],
                        ni, B, XROW, next_q())

                wb = w[:].unsqueeze(-1).to_broadcast([P, Fq, B])
                if XBF16:
                    nc.vector.tensor_tensor(out=m3, in0=gdst, in1=wb,
                                            op=mybir.AluOpType.mult)
                else:
                    nc.vector.tensor_mul(m3, m3, wb)

                if q == 0:
                    aggt = agg0
                else:
                    aggt = wp.tile([P, G * B], F32, tag="aggq")
                    aggq_tiles.append(aggt)
                for (ga, gb2, d) in _equal_d_runs(D[q], 0, int(Gact[q])):
                    if d == 0:
                        continue
                    src_ap = (m3[:, int(S[q, ga]):int(S[q, gb2]), :]
                              .rearrange("p (n d) b -> p n b d", d=d))
                    dst_ap = aggt[:, ga * B:gb2 * B].rearrange(
                        "p (n b) -> p n b", b=B)
                    nc.vector.tensor_reduce(
                        dst_ap, src_ap, axis=mybir.AxisListType.X,
                        op=mybir.AluOpType.add)

                if q > 0:
                    # scatter active rows into canonical order through HBM
                    a3 = aggt[:].rearrange("p (g b) -> p g b", b=B)
                    nact = int(Gact[q]) * P
                    half = min((G // 2) * P, nact)
                    sbase = int(sidx_bases[q])
                    nc.gpsimd.dma_scatter_add(
                        pagg[q - 1][:, :B], a3[:, :half // P, :],
                        _slice_idx(wp, nc, siD, sbase, half),
                        half, half, B, elem_step=ROWE,
                        single_packet=False, queue_num=next_q())
                    if nact > half:
                        nc.gpsimd.dma_scatter_add(
                            pagg[q - 1][:, :B],
                            a3[:, half // P:nact // P, :],
                            _slice_idx(wp, nc, siD, sbase + half // 16,
                                       nact - half),
                            nact - half, nact - half, B, elem_step=ROWE,
                            single_packet=False, queue_num=next_q())

            # ---- merge + ODE epilogue ----
            rdb = []
            for q in range(1, nq):
                if int(Gact[q]) == 0:
                    continue
                rt = ppool.tile([P, G * B], F32, tag=f"rdb{q}")
                nc.sync.dma_start(
                    out=rt[:].rearrange("p (g b) -> p g b", b=B),
                    in_=pagg[q - 1][:, :B].rearrange(
                        "(p g) b -> p g b", p=P))
                rdb.append(rt)
            for rt in rdb:
                nc.vector.tensor_add(agg0[:], agg0[:], rt[:])

            bi = ppool.tile([P, G], F32, tag="bi")
            rl = ppool.tile([P, G], F32, tag="rl")
            br = ppool.tile([P, G], F32, tag="br")
            cp = ppool.tile([P, G], F32, tag="cp")
            xo = ppool.tile([P, G * B], F32, tag="xo")
            nc.sync.dma_start(out=bi[:], in_=biD[:, :])
            nc.sync.dma_start(out=rl[:], in_=rlD[:, :])
            nc.sync.dma_start(out=br[:], in_=brD[:, :])
            nc.sync.dma_start(out=cp[:], in_=cpD[:, :])
            nc.sync.dma_start(out=xo[:], in_=xoD[:, :])

            rate = ppool.tile([P, G], F32, tag="rate")
            nc.scalar.activation(rate[:], rl[:], Exp)
            nc.vector.tensor_mul(rate[:], rate[:], br[:])

            a3 = agg0[:].rearrange("p (g b) -> p g b", b=B)
            bib = bi[:].unsqueeze(-1).to_broadcast([P, G, B])
            cpb = cp[:].unsqueeze(-1).to_broadcast([P, G, B])
            rateb = rate[:].unsqueeze(-1).to_broadcast([P, G, B])

            dr = ppool.tile([P, G * B], F32, tag="dr")
            d3 = dr[:].rearrange("p (g b) -> p g b", b=B)
            nc.vector.tensor_add(d3, a3, bib)
            nc.scalar.activation(dr[:], dr[:], Tanh)
            nc.vector.tensor_mul(d3, d3, cpb)
            nc.vector.tensor_tensor(out=dr[:], in0=dr[:], in1=xo[:],
                                    op=mybir.AluOpType.subtract)
            nc.vector.tensor_mul(d3, d3, rateb)
            nc.vector.tensor_scalar_mul(dr[:], dr[:], float(DT))
            nc.vector.tensor_add(dr[:], dr[:], xo[:])
            nc.vector.tensor_scalar_max(dr[:], dr[:], 0.0)
            nc.vector.tensor_tensor(out=d3, in0=d3, in1=cpb,
                                    op=mybir.AluOpType.min)
            nc.sync.dma_start(out=outD[:, :], in_=dr[:])

    nc.compile()
    return nc


def _slice_idx(wp, nc, siD, col0, n):
    t = wp.tile([128, (n + 15) // 16], I16, tag="sidx")
    nc.sync.dma_start(out=t[:], in_=siD[:, col0:col0 + (n + 15) // 16])
    return t[:]


# ---------------------------------------------------------------------------
# Entry point
# ---------------------------------------------------------------------------

def kernel(x, theta_graph, node_bias, rate_log_scale, base_rate, capacity,
           sign_prior, conf_scale, delay_scale, src_index, dst_index,
           learn_mask):
    ins, plan = _prep(x, theta_graph, node_bias, rate_log_scale, base_rate,
                      capacity, sign_prior, conf_scale, delay_scale,
                      src_index, dst_index, learn_mask, NCORES)
    nc = build(plan["B"], plan["N"], plan["G"], plan["nq"], plan["D"],
               plan["S"], plan["F"], plan["Scol"], plan["n_pad"],
               plan["gidx_cols"], plan["sidx_cols"], Gact=plan["Gact"])
    res = run_bass_kernel_spmd(nc, ins, core_ids=list(range(NCORES)))
    return _assemble(res.results, plan)



# revision 2
# speedup vs baseline: 1.6268x; 1.6268x over previous
"""Trainium2 Bass kernel for DifferentiableSupergraphDynamics.

Computation:
    edge_w = where(learn_mask, tanh(theta), sign*conf) * delay      [E]
    msgs   = x[:, src] * edge_w                                     [B, E]
    agg    = scatter_add(msgs -> dst)                               [B, N]
    rate   = base_rate * exp(rate_log_scale)                        [N]
    drive  = tanh(agg + bias)
    x_next = clip(x + DT * rate * (drive*cap - x), 0, cap)

Design (v2):
  - Destination nodes are dealt round-robin by total-degree rank across the
    8 cores; every edge lives on its destination's core (no collective).
  - Single-structure gather: the x table is packed as [N/4, 4*B] f32 (4 node
    rows per 256B line).  Each edge's SWDGE descriptor fetches the full 256B
    line holding its source row, so the int16 gather index (= src//4 <
    32768) reaches the whole table and no src-range structure split / merge
    scatter is needed.  Row selection happens in the weighted reduce: a
    host-built [4]-wide weight mask per slot (w at position src%4, else 0).
  - CSR: per-core nodes sorted by degree, groups of 128 partitions padded to
    the group max degree D (D shared across cores).  Whole groups are packed
    into "strips" of <= STRIP_COLS columns; one gather call per strip
    (4096-5k slots, SWDGE queues round-robin 1,2,3,0), double-buffered, with
    the DVE multiply + two-pass reduce (sub-row, then degree window) running
    under the next strips' gathers.
  - Edge weights and the epilogue's affine terms are precomputed on host:
    out = clip(A + C*tanh(agg + bias), 0, cap) with A=(1-DT*rate)*x and
    C = DT*rate*cap.
"""

import numpy as np

import concourse.bass as bass
import concourse.bacc as bacc
import concourse.mybir as mybir
import concourse.tile as tile
from concourse.bass_utils import run_bass_kernel_spmd

P = 128
NCORES = 8
DT = 0.1
EPS = 1e-5
STRIP_COLS = 32            # target strip width (cols); singleton groups may exceed
MAX_CALL = 8192            # SWDGE ring capacity per call

F32 = mybir.dt.float32
I16 = mybir.dt.int16


def _wrap_idx(flat):
    """SWDGE wrapped int16 index layout for one call: index j at
    [j%16, j//16], replicated to 128 partitions."""
    n = len(flat)
    assert n % 16 == 0
    cols = flat.reshape(n // 16, 16).T
    return np.concatenate([cols] * 8, axis=0)


# ---------------------------------------------------------------------------
# Host-side data preparation
# ---------------------------------------------------------------------------

def _prep(x, theta, bias, ratelog, baserate, cap, sign, conf, delay, src, dst,
          mask, n_cores):
    B, N = x.shape
    E = src.shape[0]

    src = np.asarray(src).astype(np.int64)
    dst = np.asarray(dst).astype(np.int64)
    x = np.asarray(x, dtype=np.float32)

    # host-computed edge weight
    w = np.where(np.asarray(mask).astype(bool),
                 np.tanh(np.asarray(theta, dtype=np.float32)),
                 np.asarray(sign, dtype=np.float32) *
                 np.asarray(conf, dtype=np.float32)) \
        * np.asarray(delay, dtype=np.float32)

    deg = np.bincount(dst, minlength=N)
    order = np.argsort(-deg, kind="stable")
    rank_of = np.empty(N, dtype=np.int64)
    rank_of[order] = np.arange(N)
    core_of = rank_of % n_cores
    pos_of = rank_of // n_cores
    npc = (N + n_cores - 1) // n_cores
    G = (npc + P - 1) // P
    nper = G * P

    # shared-over-cores group degree D[g] = max over cores of group max
    degs = np.zeros((n_cores, nper), dtype=np.int64)
    for c in range(n_cores):
        dc = deg[order[c::n_cores]]
        degs[c, :len(dc)] = dc
    D = degs.reshape(n_cores, G, P).max(axis=(0, 2))
    S = np.zeros(G + 1, dtype=np.int64)
    S[1:] = np.cumsum(D)
    F = int(S[-1])

    # strips: consecutive whole groups, greedily packed to <= STRIP_COLS
    strips = []            # (g0, g1, col0, col1)
    g0 = 0
    while g0 < G:
        g1 = g0 + 1
        while g1 < G and S[g1 + 1] - S[g0] <= STRIP_COLS:
            g1 += 1
        if S[g1] - S[g0] > 0:
            strips.append((g0, g1, int(S[g0]), int(S[g1])))
        g0 = g1
    assert all((c1 - c0) * P <= MAX_CALL for (_, _, c0, c1) in strips), \
        f"strip exceeds ring capacity: {strips}"

    # edge -> slot
    ec = core_of[dst]
    ep = pos_of[dst]
    eord = np.argsort(ec * nper + ep, kind="stable")
    key = (ec * nper + ep)[eord]
    key_change = np.ones(E, dtype=bool)
    key_change[1:] = key[1:] != key[:-1]
    run_starts = np.flatnonzero(key_change)
    occ = np.arange(E) - run_starts[np.cumsum(key_change) - 1]
    g = ep[eord] // P
    pp = ep[eord] % P
    col = S[g] + occ
    slot_i = pp + P * col

    n4 = (N + 3) // 4
    srcg = (src[eord] // 4).astype(np.int16)
    subr = (src[eord] % 4).astype(np.int64)
    assert n4 <= 32768

    idxA = np.zeros((n_cores, F * P), np.int16)
    w4A = np.zeros((n_cores, P, F, 4), np.float32)
    ecs = ec[eord]
    idxA[ecs, slot_i] = srcg
    w4A[ecs, pp, col, subr] = w[eord]

    # wrapped gather indices per strip, concatenated
    gidx = np.zeros((n_cores, 128, F * 8), np.int16)
    for (g0_, g1_, c0, c1) in strips:
        for c in range(n_cores):
            gidx[c][:, c0 * 8:c1 * 8] = _wrap_idx(idxA[c, c0 * P:c1 * P])

    # node params in canonical [P, G] placement
    rate = np.asarray(baserate, dtype=np.float32) * \
        np.exp(np.asarray(ratelog, dtype=np.float32))
    Cv = DT * rate * np.asarray(cap, dtype=np.float32)
    Av = (1.0 - DT * rate)[None, :] * x            # [B, N]

    biasA = np.zeros((n_cores, P, G), np.float32)
    CA = np.zeros((n_cores, P, G), np.float32)
    capA = np.ones((n_cores, P, G), np.float32)
    AA = np.zeros((n_cores, P, G, B), np.float32)
    node_ids = np.full((n_cores, P, G), -1, np.int64)
    biasv = np.asarray(bias, dtype=np.float32)
    capv = np.asarray(cap, dtype=np.float32)
    for c in range(n_cores):
        nd = order[c::n_cores]                     # nodes at pos 0..len-1
        j = np.arange(len(nd))
        pidx = (j % P, j // P)
        node_ids[c][pidx] = nd
        biasA[c][pidx] = biasv[nd]
        CA[c][pidx] = Cv[nd]
        capA[c][pidx] = capv[nd]
        AA[c][pidx[0], pidx[1], :] = Av[:, nd].T

    xq = np.zeros((n4, 4 * B), np.float32)
    xq.reshape(-1, B)[:N] = x.T

    ins = []
    for c in range(n_cores):
        ins.append({
            "xq": xq,
            "gidx": gidx[c],
            "w4": w4A[c].reshape(P, F * 4),
            "bias": biasA[c],
            "cmul": CA[c],
            "cap": capA[c],
            "apre": AA[c].reshape(P, G * B),
        })
    plan = dict(B=B, G=G, F=F, D=D, S=S, strips=strips, n4=n4,
                node_ids=node_ids)
    return ins, plan


def _assemble(results, plan):
    B, G = plan["B"], plan["G"]
    N = 0
    for nid in plan["node_ids"]:
        N = max(N, nid.max() + 1)
    out = np.empty((B, N), dtype=np.float32)
    for ci, res in enumerate(results):
        o = res["out"].reshape(P, G, B)
        nid = plan["node_ids"][ci]
        ok = nid >= 0
        out[:, nid[ok]] = o[ok].T
    return out


# ---------------------------------------------------------------------------
# Device kernel
# ---------------------------------------------------------------------------

def _equal_d_runs(D, g0, g1):
    runs = []
    a = g0
    while a < g1:
        b = a + 1
        while b < g1 and D[b] == D[a]:
            b += 1
        runs.append((a, b, int(D[a])))
        a = b
    return runs


def build(B, G, F, D, S, strips, n4):
    nc = bacc.Bacc("TRN2", target_bir_lowering=False, debug=False,
                   enable_asserts=False, num_swdge_queues=4)

    xqD = nc.dram_tensor("xq", [n4, 4 * B], F32, kind="ExternalInput")
    giD = nc.dram_tensor("gidx", [128, F * 8], I16, kind="ExternalInput")
    w4D = nc.dram_tensor("w4", [P, F * 4], F32, kind="ExternalInput")
    biD = nc.dram_tensor("bias", [P, G], F32, kind="ExternalInput")
    cmD = nc.dram_tensor("cmul", [P, G], F32, kind="ExternalInput")
    cpD = nc.dram_tensor("cap", [P, G], F32, kind="ExternalInput")
    apD = nc.dram_tensor("apre", [P, G * B], F32, kind="ExternalInput")
    outD = nc.dram_tensor("out", [P, G * B], F32, kind="ExternalOutput")

    Tanh = mybir.ActivationFunctionType.Tanh
    qorder = [1, 2, 3, 0]

    with tile.TileContext(nc) as tc:
        with (
            tc.tile_pool(name="persist", bufs=1) as ppool,
            tc.tile_pool(name="strip", bufs=6) as sp,
        ):
            agg = ppool.tile([P, G * B], F32, tag="agg")
            nc.vector.memset(agg[:], 0.0)

            for si, (g0, g1, c0, c1) in enumerate(strips):
                sc = c1 - c0
                nidx = sc * P
                gt = sp.tile([128, sc * 8], I16, tag="gidx")
                nc.sync.dma_start(out=gt[:], in_=giD[:, c0 * 8:c1 * 8])
                wt = sp.tile([P, sc * 4], F32, tag="w4")
                nc.sync.dma_start(out=wt[:], in_=w4D[:, c0 * 4:c1 * 4])

                msgs = sp.tile([P, sc * 4 * B], F32, tag="msgs")
                m3 = msgs[:].rearrange("p (c e) -> p c e", e=4 * B)
                nc.gpsimd.dma_gather(
                    m3, xqD[:, :], gt[:], nidx, nidx, 4 * B,
                    single_packet=False, queue_num=qorder[si % 4])

                m4 = msgs[:].rearrange("p (c s b) -> p c s b", s=4, b=B)
                w4b = (wt[:].rearrange("p (c s) -> p c s", s=4)
                       .unsqueeze(-1).to_broadcast([P, sc, 4, B]))
                nc.vector.tensor_mul(m4, m4, w4b)

                msum = sp.tile([P, sc * B], F32, tag="msum")
                ms3 = msum[:].rearrange("p (c b) -> p c b", b=B)
                nc.vector.tensor_reduce(
                    ms3, msgs[:].rearrange("p (c s b) -> p c b s", s=4, b=B),
                    axis=mybir.AxisListType.X, op=mybir.AluOpType.add)

                for (ga, gb, d) in _equal_d_runs(D, g0, g1):
                    if d == 0:
                        continue
                    src_ap = (ms3[:, int(S[ga]) - c0:int(S[gb]) - c0, :]
                              .rearrange("p (n d) b -> p n b d", d=d))
                    dst_ap = agg[:, ga * B:gb * B].rearrange(
                        "p (n b) -> p n b", b=B)
                    nc.vector.tensor_reduce(
                        dst_ap, src_ap, axis=mybir.AxisListType.X,
                        op=mybir.AluOpType.add)

            # ---- epilogue: out = clip(A + C*tanh(agg + bias), 0, cap) ----
            bi = ppool.tile([P, G], F32, tag="bi")
            cm = ppool.tile([P, G], F32, tag="cm")
            cp = ppool.tile([P, G], F32, tag="cp")
            ap_ = ppool.tile([P, G * B], F32, tag="ap")
            nc.sync.dma_start(out=bi[:], in_=biD[:, :])
            nc.sync.dma_start(out=cm[:], in_=cmD[:, :])
            nc.sync.dma_start(out=cp[:], in_=cpD[:, :])
            nc.sync.dma_start(out=ap_[:], in_=apD[:, :])

            a3 = agg[:].rearrange("p (g b) -> p g b", b=B)
            bib = bi[:].unsqueeze(-1).to_broadcast([P, G, B])
            cmb = cm[:].unsqueeze(-1).to_broadcast([P, G, B])
            cpb = cp[:].unsqueeze(-1).to_broadcast([P, G, B])

            nc.vector.tensor_add(a3, a3, bib)
            nc.scalar.activation(agg[:], agg[:], Tanh)
            nc.vector.tensor_mul(a3, a3, cmb)
            nc.vector.tensor_add(agg[:], agg[:], ap_[:])
            nc.vector.tensor_scalar_max(agg[:], agg[:], 0.0)
            nc.vector.tensor_tensor(out=a3, in0=a3, in1=cpb,
                                    op=mybir.AluOpType.min)
            nc.sync.dma_start(out=outD[:, :], in_=agg[:])

    nc.compile()
    return nc


# ---------------------------------------------------------------------------
# Entry point
# ---------------------------------------------------------------------------

def kernel(x, theta_graph, node_bias, rate_log_scale, base_rate, capacity,
           sign_prior, conf_scale, delay_scale, src_index, dst_index,
           learn_mask):
    ins, plan = _prep(x, theta_graph, node_bias, rate_log_scale, base_rate,
                      capacity, sign_prior, conf_scale, delay_scale,
                      src_index, dst_index, learn_mask, NCORES)
    nc = build(plan["B"], plan["G"], plan["F"], plan["D"], plan["S"],
               plan["strips"], plan["n4"])
    res = run_bass_kernel_spmd(nc, ins, core_ids=list(range(NCORES)))
    return _assemble(res.results, plan)


# revision 4
# speedup vs baseline: 1.6715x; 1.0275x over previous
"""Trainium2 Bass kernel for DifferentiableSupergraphDynamics.

Computation:
    edge_w = where(learn_mask, tanh(theta), sign*conf) * delay      [E]
    msgs   = x[:, src] * edge_w                                     [B, E]
    agg    = scatter_add(msgs -> dst)                               [B, N]
    rate   = base_rate * exp(rate_log_scale)                        [N]
    drive  = tanh(agg + bias)
    x_next = clip(x + DT * rate * (drive*cap - x), 0, cap)

Design (v2):
  - Destination nodes are dealt round-robin by total-degree rank across the
    8 cores; every edge lives on its destination's core (no collective).
  - Single-structure gather: the x table is packed as [N/4, 4*B] f32 (4 node
    rows per 256B line).  Each edge's SWDGE descriptor fetches the full 256B
    line holding its source row, so the int16 gather index (= src//4 <
    32768) reaches the whole table and no src-range structure split / merge
    scatter is needed.  Row selection happens in the weighted reduce: a
    host-built [4]-wide weight mask per slot (w at position src%4, else 0).
  - CSR: per-core nodes sorted by degree, groups of 128 partitions padded to
    the group max degree D (D shared across cores).  Whole groups are packed
    into "strips" of <= STRIP_COLS columns; one gather call per strip
    (4096-5k slots, SWDGE queues round-robin 1,2,3,0), double-buffered, with
    the DVE multiply + two-pass reduce (sub-row, then degree window) running
    under the next strips' gathers.
  - Edge weights and the epilogue's affine terms are precomputed on host:
    out = clip(A + C*tanh(agg + bias), 0, cap) with A=(1-DT*rate)*x and
    C = DT*rate*cap.
"""

import numpy as np

import concourse.bass as bass
import concourse.bacc as bacc
import concourse.mybir as mybir
import concourse.tile as tile
from concourse.bass_utils import run_bass_kernel_spmd

P = 128
NCORES = 8
DT = 0.1
EPS = 1e-5
STRIP_COLS = 32            # target strip width (cols); singleton groups may exceed
MAX_CALL = 8192            # SWDGE ring capacity per call

F32 = mybir.dt.float32
I16 = mybir.dt.int16


def _wrap_idx(flat):
    """SWDGE wrapped int16 index layout for one call: index j at
    [j%16, j//16], replicated to 128 partitions."""
    n = len(flat)
    assert n % 16 == 0
    cols = flat.reshape(n // 16, 16).T
    return np.concatenate([cols] * 8, axis=0)


# ---------------------------------------------------------------------------
# Host-side data preparation
# ---------------------------------------------------------------------------

def _prep(x, theta, bias, ratelog, baserate, cap, sign, conf, delay, src, dst,
          mask, n_cores):
    B, N = x.shape
    E = src.shape[0]

    src = np.asarray(src).astype(np.int64)
    dst = np.asarray(dst).astype(np.int64)
    x = np.asarray(x, dtype=np.float32)

    # host-computed edge weight
    w = np.where(np.asarray(mask).astype(bool),
                 np.tanh(np.asarray(theta, dtype=np.float32)),
                 np.asarray(sign, dtype=np.float32) *
                 np.asarray(conf, dtype=np.float32)) \
        * np.asarray(delay, dtype=np.float32)

    deg = np.bincount(dst, minlength=N)
    order = np.argsort(-deg, kind="stable")
    rank_of = np.empty(N, dtype=np.int64)
    rank_of[order] = np.arange(N)
    core_of = rank_of % n_cores
    pos_of = rank_of // n_cores
    npc = (N + n_cores - 1) // n_cores
    G = (npc + P - 1) // P
    nper = G * P

    # shared-over-cores group degree D[g] = max over cores of group max
    degs = np.zeros((n_cores, nper), dtype=np.int64)
    for c in range(n_cores):
        dc = deg[order[c::n_cores]]
        degs[c, :len(dc)] = dc
    D = degs.reshape(n_cores, G, P).max(axis=(0, 2))
    S = np.zeros(G + 1, dtype=np.int64)
    S[1:] = np.cumsum(D)
    F = int(S[-1])

    # strips: consecutive whole groups, greedily packed to <= STRIP_COLS
    strips = []            # (g0, g1, col0, col1)
    g0 = 0
    while g0 < G:
        g1 = g0 + 1
        while g1 < G and S[g1 + 1] - S[g0] <= STRIP_COLS:
            g1 += 1
        if S[g1] - S[g0] > 0:
            strips.append((g0, g1, int(S[g0]), int(S[g1])))
        g0 = g1
    assert all((c1 - c0) * P <= MAX_CALL for (_, _, c0, c1) in strips), \
        f"strip exceeds ring capacity: {strips}"
    # schedule: queue 0's ucode blocks the Pool pipeline for its full
    # generation time, so hand it the smallest strips (every 4th slot).
    by_size = sorted(strips, key=lambda s: s[3] - s[2], reverse=True)
    nq0 = len(strips) // 4
    big, small = by_size[:len(strips) - nq0], by_size[len(strips) - nq0:]
    sched = []
    bi_, si_ = 0, 0
    for i in range(len(strips)):
        if i % 4 == 3 and si_ < len(small):
            sched.append(small[si_]); si_ += 1
        elif bi_ < len(big):
            sched.append(big[bi_]); bi_ += 1
        else:
            sched.append(small[si_]); si_ += 1
    strips = sched

    # edge -> slot
    ec = core_of[dst]
    ep = pos_of[dst]
    eord = np.argsort(ec * nper + ep, kind="stable")
    key = (ec * nper + ep)[eord]
    key_change = np.ones(E, dtype=bool)
    key_change[1:] = key[1:] != key[:-1]
    run_starts = np.flatnonzero(key_change)
    occ = np.arange(E) - run_starts[np.cumsum(key_change) - 1]
    g = ep[eord] // P
    pp = ep[eord] % P
    col = S[g] + occ
    slot_i = pp + P * col

    n4 = (N + 3) // 4
    srcg = (src[eord] // 4).astype(np.int16)
    subr = (src[eord] % 4).astype(np.int64)
    assert n4 <= 32768

    idxA = np.zeros((n_cores, F * P), np.int16)
    w4A = np.zeros((n_cores, P, F, 4), np.float32)
    ecs = ec[eord]
    idxA[ecs, slot_i] = srcg
    w4A[ecs, pp, col, subr] = w[eord]

    # wrapped gather indices per strip, concatenated
    gidx = np.zeros((n_cores, 128, F * 8), np.int16)
    for (g0_, g1_, c0, c1) in strips:
        for c in range(n_cores):
            gidx[c][:, c0 * 8:c1 * 8] = _wrap_idx(idxA[c, c0 * P:c1 * P])

    # node params in canonical [P, G] placement
    rate = np.asarray(baserate, dtype=np.float32) * \
        np.exp(np.asarray(ratelog, dtype=np.float32))
    Cv = DT * rate * np.asarray(cap, dtype=np.float32)
    Av = (1.0 - DT * rate)[None, :] * x            # [B, N]

    biasA = np.zeros((n_cores, P, G), np.float32)
    CA = np.zeros((n_cores, P, G), np.float32)
    capA = np.ones((n_cores, P, G), np.float32)
    AA = np.zeros((n_cores, P, G, B), np.float32)
    node_ids = np.full((n_cores, P, G), -1, np.int64)
    biasv = np.asarray(bias, dtype=np.float32)
    capv = np.asarray(cap, dtype=np.float32)
    for c in range(n_cores):
        nd = order[c::n_cores]                     # nodes at pos 0..len-1
        j = np.arange(len(nd))
        pidx = (j % P, j // P)
        node_ids[c][pidx] = nd
        biasA[c][pidx] = biasv[nd]
        CA[c][pidx] = Cv[nd]
        capA[c][pidx] = capv[nd]
        AA[c][pidx[0], pidx[1], :] = Av[:, nd].T

    xq = np.zeros((n4, 4 * B), np.float32)
    xq.reshape(-1, B)[:N] = x.T

    ins = []
    for c in range(n_cores):
        ins.append({
            "xq": xq,
            "gidx": gidx[c],
            "w4": w4A[c].reshape(P, F * 4),
            "bias": biasA[c],
            "cmul": CA[c],
            "cap": capA[c],
            "apre": AA[c].reshape(P, G * B),
        })
    plan = dict(B=B, G=G, F=F, D=D, S=S, strips=strips, n4=n4,
                node_ids=node_ids)
    return ins, plan


def _assemble(results, plan):
    B, G = plan["B"], plan["G"]
    N = 0
    for nid in plan["node_ids"]:
        N = max(N, nid.max() + 1)
    out = np.empty((B, N), dtype=np.float32)
    for ci, res in enumerate(results):
        o = res["out"].reshape(P, G, B)
        nid = plan["node_ids"][ci]
        ok = nid >= 0
        out[:, nid[ok]] = o[ok].T
    return out


# ---------------------------------------------------------------------------
# Device kernel
# ---------------------------------------------------------------------------

def _equal_d_runs(D, g0, g1):
    runs = []
    a = g0
    while a < g1:
        b = a + 1
        while b < g1 and D[b] == D[a]:
            b += 1
        runs.append((a, b, int(D[a])))
        a = b
    return runs


def build(B, G, F, D, S, strips, n4):
    nc = bacc.Bacc("TRN2", target_bir_lowering=False, debug=False,
                   enable_asserts=False, num_swdge_queues=4)

    xqD = nc.dram_tensor("xq", [n4, 4 * B], F32, kind="ExternalInput")
    giD = nc.dram_tensor("gidx", [128, F * 8], I16, kind="ExternalInput")
    w4D = nc.dram_tensor("w4", [P, F * 4], F32, kind="ExternalInput")
    biD = nc.dram_tensor("bias", [P, G], F32, kind="ExternalInput")
    cmD = nc.dram_tensor("cmul", [P, G], F32, kind="ExternalInput")
    cpD = nc.dram_tensor("cap", [P, G], F32, kind="ExternalInput")
    apD = nc.dram_tensor("apre", [P, G * B], F32, kind="ExternalInput")
    outD = nc.dram_tensor("out", [P, G * B], F32, kind="ExternalOutput")

    Tanh = mybir.ActivationFunctionType.Tanh
    qorder = [1, 2, 3, 0]

    with tile.TileContext(nc) as tc:
        with (
            tc.tile_pool(name="persist", bufs=1) as ppool,
            tc.tile_pool(name="strip", bufs=8) as sp,
        ):
            agg = ppool.tile([P, G * B], F32, tag="agg")
            nc.vector.memset(agg[:], 0.0)

            # epilogue params up front so the tail never waits on DMA
            bi = ppool.tile([P, G], F32, tag="bi")
            cm = ppool.tile([P, G], F32, tag="cm")
            cp = ppool.tile([P, G], F32, tag="cp")
            ap_ = ppool.tile([P, G * B], F32, tag="ap")

            for si, (g0, g1, c0, c1) in enumerate(strips):
                sc = c1 - c0
                nidx = sc * P
                gt = sp.tile([128, sc * 8], I16, tag="gidx")
                nc.sync.dma_start(out=gt[:], in_=giD[:, c0 * 8:c1 * 8])
                wt = sp.tile([P, sc * 4], F32, tag="w4")
                nc.sync.dma_start(out=wt[:], in_=w4D[:, c0 * 4:c1 * 4])
                if si == 0:
                    nc.sync.dma_start(out=bi[:], in_=biD[:, :])
                    nc.sync.dma_start(out=cm[:], in_=cmD[:, :])
                    nc.sync.dma_start(out=cp[:], in_=cpD[:, :])
                    nc.sync.dma_start(out=ap_[:], in_=apD[:, :])

                msgs = sp.tile([P, sc * 4 * B], F32, tag="msgs")
                m3 = msgs[:].rearrange("p (c e) -> p c e", e=4 * B)
                nc.gpsimd.dma_gather(
                    m3, xqD[:, :], gt[:], nidx, nidx, 4 * B,
                    single_packet=False, queue_num=qorder[si % 4])

                m2 = msgs[:].rearrange("p (q b) -> p q b", b=B)
                w4b = wt[:].unsqueeze(-1).to_broadcast([P, sc * 4, B])
                nc.vector.tensor_mul(m2, m2, w4b)

                # fused (sub-row x degree-window) reduce: for a run of
                # groups with equal degree d, each group's 4*d*B block is
                # contiguous with uniform stride B over the (d,s) axis.
                for (ga, gb, d) in _equal_d_runs(D, g0, g1):
                    if d == 0:
                        continue
                    src_ap = (msgs[:, (int(S[ga]) - c0) * 4 * B:
                              (int(S[gb]) - c0) * 4 * B]
                              .rearrange("p (n dd b) -> p n b dd",
                                         dd=4 * d, b=B))
                    dst_ap = agg[:, ga * B:gb * B].rearrange(
                        "p (n b) -> p n b", b=B)
                    nc.vector.tensor_reduce(
                        dst_ap, src_ap, axis=mybir.AxisListType.X,
                        op=mybir.AluOpType.add)

            # ---- epilogue: out = clip(A + C*tanh(agg + bias), 0, cap) ----

            a3 = agg[:].rearrange("p (g b) -> p g b", b=B)
            bib = bi[:].unsqueeze(-1).to_broadcast([P, G, B])
            cmb = cm[:].unsqueeze(-1).to_broadcast([P, G, B])
            cpb = cp[:].unsqueeze(-1).to_broadcast([P, G, B])

            nc.vector.tensor_add(a3, a3, bib)
            nc.scalar.activation(agg[:], agg[:], Tanh)
            nc.vector.tensor_mul(a3, a3, cmb)
            nc.vector.tensor_add(agg[:], agg[:], ap_[:])
            nc.vector.tensor_scalar_max(agg[:], agg[:], 0.0)
            nc.vector.tensor_tensor(out=a3, in0=a3, in1=cpb,
                                    op=mybir.AluOpType.min)
            nc.sync.dma_start(out=outD[:, :], in_=agg[:])

    nc.compile()
    return nc


# ---------------------------------------------------------------------------
# Entry point
# ---------------------------------------------------------------------------

def kernel(x, theta_graph, node_bias, rate_log_scale, base_rate, capacity,
           sign_prior, conf_scale, delay_scale, src_index, dst_index,
           learn_mask):
    ins, plan = _prep(x, theta_graph, node_bias, rate_log_scale, base_rate,
                      capacity, sign_prior, conf_scale, delay_scale,
                      src_index, dst_index, learn_mask, NCORES)
    nc = build(plan["B"], plan["G"], plan["F"], plan["D"], plan["S"],
               plan["strips"], plan["n4"])
    res = run_bass_kernel_spmd(nc, ins, core_ids=list(range(NCORES)))
    return _assemble(res.results, plan)


# revision 6
# speedup vs baseline: 1.7460x; 1.0445x over previous
"""Trainium2 Bass kernel for DifferentiableSupergraphDynamics.

Computation:
    edge_w = where(learn_mask, tanh(theta), sign*conf) * delay      [E]
    msgs   = x[:, src] * edge_w                                     [B, E]
    agg    = scatter_add(msgs -> dst)                               [B, N]
    rate   = base_rate * exp(rate_log_scale)                        [N]
    drive  = tanh(agg + bias)
    x_next = clip(x + DT * rate * (drive*cap - x), 0, cap)

Design (v2):
  - Destination nodes are dealt round-robin by total-degree rank across the
    8 cores; every edge lives on its destination's core (no collective).
  - Single-structure gather: the x table is packed as [N/4, 4*B] f32 (4 node
    rows per 256B line).  Each edge's SWDGE descriptor fetches the full 256B
    line holding its source row, so the int16 gather index (= src//4 <
    32768) reaches the whole table and no src-range structure split / merge
    scatter is needed.  Row selection happens in the weighted reduce: a
    host-built [4]-wide weight mask per slot (w at position src%4, else 0).
  - CSR: per-core nodes sorted by degree, groups of 128 partitions padded to
    the group max degree D (D shared across cores).  Whole groups are packed
    into "strips" of <= STRIP_COLS columns; one gather call per strip
    (4096-5k slots, SWDGE queues round-robin 1,2,3,0), double-buffered, with
    the DVE multiply + two-pass reduce (sub-row, then degree window) running
    under the next strips' gathers.
  - Edge weights and the epilogue's affine terms are precomputed on host:
    out = clip(A + C*tanh(agg + bias), 0, cap) with A=(1-DT*rate)*x and
    C = DT*rate*cap.
"""

import numpy as np

import concourse.bass as bass
import concourse.bacc as bacc
import concourse.mybir as mybir
import concourse.tile as tile
from concourse.bass_utils import run_bass_kernel_spmd

P = 128
NCORES = 8
DT = 0.1
EPS = 1e-5
STRIP_COLS = 24            # slab width in columns (3072-slot gather calls)
MAX_CALL = 8192            # SWDGE ring capacity per call

F32 = mybir.dt.float32
I16 = mybir.dt.int16


def _wrap_idx(flat):
    """SWDGE wrapped int16 index layout for one call: index j at
    [j%16, j//16], replicated to 128 partitions."""
    n = len(flat)
    assert n % 16 == 0
    cols = flat.reshape(n // 16, 16).T
    return np.concatenate([cols] * 8, axis=0)


# ---------------------------------------------------------------------------
# Host-side data preparation
# ---------------------------------------------------------------------------

def _prep(x, theta, bias, ratelog, baserate, cap, sign, conf, delay, src, dst,
          mask, n_cores):
    B, N = x.shape
    E = src.shape[0]

    src = np.asarray(src).astype(np.int64)
    dst = np.asarray(dst).astype(np.int64)
    x = np.asarray(x, dtype=np.float32)

    # host-computed edge weight
    w = np.where(np.asarray(mask).astype(bool),
                 np.tanh(np.asarray(theta, dtype=np.float32)),
                 np.asarray(sign, dtype=np.float32) *
                 np.asarray(conf, dtype=np.float32)) \
        * np.asarray(delay, dtype=np.float32)

    deg = np.bincount(dst, minlength=N)
    order = np.argsort(-deg, kind="stable")
    rank_of = np.empty(N, dtype=np.int64)
    rank_of[order] = np.arange(N)
    core_of = rank_of % n_cores
    pos_of = rank_of // n_cores
    npc = (N + n_cores - 1) // n_cores
    G = (npc + P - 1) // P
    nper = G * P

    # shared-over-cores group degree D[g] = max over cores of group max
    degs = np.zeros((n_cores, nper), dtype=np.int64)
    for c in range(n_cores):
        dc = deg[order[c::n_cores]]
        degs[c, :len(dc)] = dc
    D = degs.reshape(n_cores, G, P).max(axis=(0, 2))
    S = np.zeros(G + 1, dtype=np.int64)
    S[1:] = np.cumsum(D)
    F = int(S[-1])

    # uniform slabs of SLAB_COLS columns (cut anywhere; group windows that
    # straddle a slab boundary are reduced partially and accumulated)
    slabs = []
    A = 0
    while A < F:
        Bc = min(A + STRIP_COLS, F)
        runs = []          # (ga, gb, d): full windows, equal-d batches
        partials = []      # (g, a, b): partial window cols [a, b)
        fulls = []
        for g in range(G):
            if D[g] == 0 or S[g] >= Bc or S[g + 1] <= A:
                continue
            a, b = max(int(S[g]), A), min(int(S[g + 1]), Bc)
            if a == S[g] and b == S[g + 1]:
                fulls.append(g)
            else:
                partials.append((g, a, b))
        i = 0
        while i < len(fulls):
            j = i + 1
            while (j < len(fulls) and fulls[j] == fulls[j - 1] + 1
                   and D[fulls[j]] == D[fulls[i]]):
                j += 1
            runs.append((fulls[i], fulls[j - 1] + 1, int(D[fulls[i]])))
            i = j
        slabs.append(dict(A=A, B=Bc, runs=runs, partials=partials))
        A = Bc

    # edge -> slot
    ec = core_of[dst]
    ep = pos_of[dst]
    eord = np.argsort(ec * nper + ep, kind="stable")
    key = (ec * nper + ep)[eord]
    key_change = np.ones(E, dtype=bool)
    key_change[1:] = key[1:] != key[:-1]
    run_starts = np.flatnonzero(key_change)
    occ = np.arange(E) - run_starts[np.cumsum(key_change) - 1]
    g = ep[eord] // P
    pp = ep[eord] % P
    col = S[g] + occ
    slot_i = pp + P * col

    n4 = (N + 3) // 4
    srcg = (src[eord] // 4).astype(np.int16)
    subr = (src[eord] % 4).astype(np.int64)
    assert n4 <= 32768

    idxA = np.zeros((n_cores, F * P), np.int16)
    w4A = np.zeros((n_cores, P, F, 4), np.float32)
    ecs = ec[eord]
    idxA[ecs, slot_i] = srcg
    w4A[ecs, pp, col, subr] = w[eord]

    # wrapped gather indices (whole array; slab slices are column slices)
    gidx = np.zeros((n_cores, 128, F * 8), np.int16)
    for c in range(n_cores):
        gidx[c] = _wrap_idx(idxA[c])

    # node params in canonical [P, G] placement
    rate = np.asarray(baserate, dtype=np.float32) * \
        np.exp(np.asarray(ratelog, dtype=np.float32))
    Cv = DT * rate * np.asarray(cap, dtype=np.float32)
    Av = (1.0 - DT * rate)[None, :] * x            # [B, N]

    biasA = np.zeros((n_cores, P, G), np.float32)
    CA = np.zeros((n_cores, P, G), np.float32)
    capA = np.ones((n_cores, P, G), np.float32)
    AA = np.zeros((n_cores, P, G, B), np.float32)
    node_ids = np.full((n_cores, P, G), -1, np.int64)
    biasv = np.asarray(bias, dtype=np.float32)
    capv = np.asarray(cap, dtype=np.float32)
    for c in range(n_cores):
        nd = order[c::n_cores]                     # nodes at pos 0..len-1
        j = np.arange(len(nd))
        pidx = (j % P, j // P)
        node_ids[c][pidx] = nd
        biasA[c][pidx] = biasv[nd]
        CA[c][pidx] = Cv[nd]
        capA[c][pidx] = capv[nd]
        AA[c][pidx[0], pidx[1], :] = Av[:, nd].T

    xq = np.zeros((n4, 4 * B), np.float32)
    xq.reshape(-1, B)[:N] = x.T

    ins = []
    for c in range(n_cores):
        ins.append({
            "xq": xq,
            "gidx": gidx[c],
            "w4": w4A[c].reshape(P, F * 4),
            "bias": biasA[c],
            "cmul": CA[c],
            "cap": capA[c],
            "apre": AA[c].reshape(P, G * B),
        })
    plan = dict(B=B, G=G, F=F, D=D, S=S, slabs=slabs, n4=n4,
                node_ids=node_ids)
    return ins, plan


def _assemble(results, plan):
    B, G = plan["B"], plan["G"]
    N = 0
    for nid in plan["node_ids"]:
        N = max(N, nid.max() + 1)
    out = np.empty((B, N), dtype=np.float32)
    for ci, res in enumerate(results):
        o = res["out"].reshape(P, G, B)
        nid = plan["node_ids"][ci]
        ok = nid >= 0
        out[:, nid[ok]] = o[ok].T
    return out


# ---------------------------------------------------------------------------
# Device kernel
# ---------------------------------------------------------------------------

def _equal_d_runs(D, g0, g1):
    runs = []
    a = g0
    while a < g1:
        b = a + 1
        while b < g1 and D[b] == D[a]:
            b += 1
        runs.append((a, b, int(D[a])))
        a = b
    return runs


def build(B, G, F, D, S, slabs, n4):
    nc = bacc.Bacc("TRN2", target_bir_lowering=False, debug=False,
                   enable_asserts=False, num_swdge_queues=4)

    xqD = nc.dram_tensor("xq", [n4, 4 * B], F32, kind="ExternalInput")
    giD = nc.dram_tensor("gidx", [128, F * 8], I16, kind="ExternalInput")
    w4D = nc.dram_tensor("w4", [P, F * 4], F32, kind="ExternalInput")
    biD = nc.dram_tensor("bias", [P, G], F32, kind="ExternalInput")
    cmD = nc.dram_tensor("cmul", [P, G], F32, kind="ExternalInput")
    cpD = nc.dram_tensor("cap", [P, G], F32, kind="ExternalInput")
    apD = nc.dram_tensor("apre", [P, G * B], F32, kind="ExternalInput")
    outD = nc.dram_tensor("out", [P, G * B], F32, kind="ExternalOutput")

    Tanh = mybir.ActivationFunctionType.Tanh
    qorder = [1, 2, 3, 0]

    with tile.TileContext(nc) as tc:
        with (
            tc.tile_pool(name="persist", bufs=1) as ppool,
            tc.tile_pool(name="strip", bufs=8) as sp,
        ):
            agg = ppool.tile([P, G * B], F32, tag="agg")
            nc.vector.memset(agg[:], 0.0)

            # epilogue params up front so the tail never waits on DMA
            bi = ppool.tile([P, G], F32, tag="bi")
            cm = ppool.tile([P, G], F32, tag="cm")
            cp = ppool.tile([P, G], F32, tag="cp")
            ap_ = ppool.tile([P, G * B], F32, tag="ap")

            for si, sl in enumerate(slabs):
                A, Bc = sl["A"], sl["B"]
                sc = Bc - A
                nidx = sc * P
                gt = sp.tile([128, sc * 8], I16, tag="gidx")
                nc.sync.dma_start(out=gt[:], in_=giD[:, A * 8:Bc * 8])
                wt = sp.tile([P, sc * 4], F32, tag="w4")
                nc.sync.dma_start(out=wt[:], in_=w4D[:, A * 4:Bc * 4])
                if si == 0:
                    nc.sync.dma_start(out=bi[:], in_=biD[:, :])
                    nc.sync.dma_start(out=cm[:], in_=cmD[:, :])
                    nc.sync.dma_start(out=cp[:], in_=cpD[:, :])
                    nc.sync.dma_start(out=ap_[:], in_=apD[:, :])

                msgs = sp.tile([P, sc * 4 * B], F32, tag="msgs")
                m3 = msgs[:].rearrange("p (c e) -> p c e", e=4 * B)
                nc.gpsimd.dma_gather(
                    m3, xqD[:, :], gt[:], nidx, nidx, 4 * B,
                    single_packet=False, queue_num=qorder[si % 4])

                m2 = msgs[:].rearrange("p (q b) -> p q b", b=B)
                w4b = wt[:].unsqueeze(-1).to_broadcast([P, sc * 4, B])
                nc.vector.tensor_mul(m2, m2, w4b)

                # fused (sub-row x degree-window) reduce: a group's window
                # block is contiguous with uniform stride B over (d, s).
                for (ga, gb, d) in sl["runs"]:
                    src_ap = (msgs[:, (int(S[ga]) - A) * 4 * B:
                              (int(S[gb]) - A) * 4 * B]
                              .rearrange("p (n dd b) -> p n b dd",
                                         dd=4 * d, b=B))
                    dst_ap = agg[:, ga * B:gb * B].rearrange(
                        "p (n b) -> p n b", b=B)
                    nc.vector.tensor_reduce(
                        dst_ap, src_ap, axis=mybir.AxisListType.X,
                        op=mybir.AluOpType.add)
                for (g, a, b) in sl["partials"]:
                    tmp = sp.tile([P, B], F32, tag="ptmp")
                    src_ap = (msgs[:, (a - A) * 4 * B:(b - A) * 4 * B]
                              .rearrange("p (n dd b) -> p n b dd",
                                         dd=4 * (b - a), b=B))
                    nc.vector.tensor_reduce(
                        tmp[:].rearrange("p (n b) -> p n b", b=B), src_ap,
                        axis=mybir.AxisListType.X, op=mybir.AluOpType.add)
                    aslice = agg[:, g * B:(g + 1) * B]
                    nc.vector.tensor_add(aslice, aslice, tmp[:])

            # ---- epilogue: out = clip(A + C*tanh(agg + bias), 0, cap) ----

            a3 = agg[:].rearrange("p (g b) -> p g b", b=B)
            bib = bi[:].unsqueeze(-1).to_broadcast([P, G, B])
            cmb = cm[:].unsqueeze(-1).to_broadcast([P, G, B])
            cpb = cp[:].unsqueeze(-1).to_broadcast([P, G, B])

            nc.vector.tensor_add(a3, a3, bib)
            nc.scalar.activation(agg[:], agg[:], Tanh)
            nc.vector.tensor_mul(a3, a3, cmb)
            nc.vector.tensor_add(agg[:], agg[:], ap_[:])
            nc.vector.tensor_scalar_max(agg[:], agg[:], 0.0)
            nc.vector.tensor_tensor(out=a3, in0=a3, in1=cpb,
                                    op=mybir.AluOpType.min)
            nc.sync.dma_start(out=outD[:, :], in_=agg[:])

    nc.compile()
    return nc


# ---------------------------------------------------------------------------
# Entry point
# ---------------------------------------------------------------------------

def kernel(x, theta_graph, node_bias, rate_log_scale, base_rate, capacity,
           sign_prior, conf_scale, delay_scale, src_index, dst_index,
           learn_mask):
    ins, plan = _prep(x, theta_graph, node_bias, rate_log_scale, base_rate,
                      capacity, sign_prior, conf_scale, delay_scale,
                      src_index, dst_index, learn_mask, NCORES)
    nc = build(plan["B"], plan["G"], plan["F"], plan["D"], plan["S"],
               plan["slabs"], plan["n4"])
    res = run_bass_kernel_spmd(nc, ins, core_ids=list(range(NCORES)))
    return _assemble(res.results, plan)
